# revision 6
# baseline (speedup 1.0000x reference)
"""Paged GQA decode attention (1 token/seq) on 8 trn2 NeuronCores.

Sharding: tensor-parallel over the 8 KV heads. Core i owns KV head i and
its G=4 query heads: Wq/Wk/Wv column-sharded, Wo row-sharded; each core
produces a partial [BS, HID] output and the host sums the 8 partials.

v3: all matmul operands in bf16 (half the HBM traffic of f32); K packed
on host as [d, tok] and V as [tok, blk, d] so every partition reads one
contiguous run; KV streamed in ~7 group DMAs (first-fit-decreasing over
sequences, small final group to shorten the tail) with triple
buffering; V stream + patches ride the ACT HWDGE ring, K + weights the
SP ring, so both descriptor generators stay busy; one exp ACT per group
over the whole [128, 4*nbg] score tile.

Per-core dataflow (one Bass program, SPMD over cores via per-core inputs):
  A) kv = hidT.T @ Wkv then q = hidT.T @ Wq (PE, 32 k-tiles in PSUM);
     RMSNorm + RoPE on [BS, D] tiles (ACT/DVE), attn scale folded into
     q's inverse-rms; k path runs first so the cache patches unblock.
  B) per group of sequences (blocks packed contiguously):
     one K DMA [128d, nbg*128] + one V DMA [128tok, nbg*130];
     new-token fix: stale cache slot at pos=T-1 gets its KT column /
     V row overwritten with this step's k/v.
     S^T[tok, 4*nbg] accumulated per block (lhsT=K block, rhs=qT_b),
     one exp over the group, per-seq tail mask,
     per-seq O^T[4, 130] += matmul(lhsT=e_blk, rhs=V_blk) with the V
     ones-column giving the softmax denominator.
  C) partial = attn^T.T @ Wo_slice (PE), DMA'd out per 512-col chunk.
"""

import numpy as np
import ml_dtypes

BF16 = ml_dtypes.bfloat16

HID, H, HKV, D = 4096, 32, 8, 128
BS, BLOCKS_PER_SEQ, BLOCK_SIZE = 32, 32, 128
G = H // HKV
EPS = 1e-6
NCORES = 8
KTILES = HID // 128  # contraction tiles for the projections
VW = D + 2  # V row width: 128 values + ones col (denominator) + pad
GMAX = 80  # max cache blocks per streamed group

_prog_cache = {}


def _plan(nb):
    """Pack whole sequences into groups of <= GMAX blocks.

    First-fit decreasing, plus one deliberately tiny final group (the
    sequence closest to 8 blocks) so the post-stream tail is short.
    Returns (pack_order, groups); groups are (start_block, [seq ids])."""
    idx = sorted(range(len(nb)), key=lambda b: -nb[b])
    fin = min(idx, key=lambda b: abs(nb[b] - 8))
    bins = []  # [total_blocks, [seq ids]]
    for b in idx:
        if b == fin:
            continue
        for bin_ in bins:
            if bin_[0] + nb[b] <= GMAX:
                bin_[0] += nb[b]
                bin_[1].append(b)
                break
        else:
            bins.append([nb[b], [b]])
    bins.append([nb[fin], [fin]])
    groups, order, start = [], [], 0
    for tot, seqs in bins:
        groups.append((start, seqs))
        order.extend(seqs)
        start += tot
    return order, groups


def _build_program(seq_lens, apply_qw, apply_kw):
    import concourse.bass as bass
    import concourse.tile as tile
    from concourse import bacc, mybir
    from concourse.masks import make_identity

    f32 = mybir.dt.float32
    bf16 = mybir.dt.bfloat16
    AF = mybir.ActivationFunctionType

    nb = [(int(t) + BLOCK_SIZE - 1) // BLOCK_SIZE for t in seq_lens]
    nbtot = sum(nb)
    _, groups = _plan(nb)

    nc = bacc.Bacc("TRN2", target_bir_lowering=False)
    hidT = nc.dram_tensor("hidT", [128, KTILES * BS], bf16, kind="ExternalInput")
    wq = nc.dram_tensor("wq", [128, KTILES * G * D], bf16, kind="ExternalInput")
    wkv = nc.dram_tensor("wkv", [128, KTILES * 2 * D], bf16, kind="ExternalInput")
    wo = nc.dram_tensor("wo", [128, G * HID], bf16, kind="ExternalInput")
    cssn = nc.dram_tensor("cssn", [BS, 2 * D], f32, kind="ExternalInput")
    kp = nc.dram_tensor("kp", [128, nbtot * BLOCK_SIZE], bf16, kind="ExternalInput")
    vp = nc.dram_tensor("vp", [128, nbtot * VW], bf16, kind="ExternalInput")
    maskd = nc.dram_tensor("maskd", [128, 128], f32, kind="ExternalInput")
    if apply_qw:
        qw = nc.dram_tensor("qw", [1, D], f32, kind="ExternalInput")
    if apply_kw:
        kw = nc.dram_tensor("kw", [1, D], f32, kind="ExternalInput")
    outp = nc.dram_tensor("outp", [BS, HID], f32, kind="ExternalOutput")

    with tile.TileContext(nc) as tc:
        with tc.tile_pool(name="sb", bufs=1) as sb, tc.tile_pool(
            name="smalls", bufs=4
        ) as smalls:
            # SP-ring DMAs first: the phase-A critical path
            hid_sb = sb.tile([128, KTILES * BS], bf16, name="hid_sb")
            nc.sync.dma_start(out=hid_sb, in_=hidT[:, :])
            wkv_sb = sb.tile([128, KTILES * 2 * D], bf16, name="wkv_sb")
            nc.sync.dma_start(out=wkv_sb, in_=wkv[:, :])
            wq_sb = sb.tile([128, KTILES * G * D], bf16, name="wq_sb", tag="bigw")
            nc.sync.dma_start(out=wq_sb, in_=wq[:, :])
            # maskM[p, c] = 1.0 if p < c else 0.0 (tail masks for all Tmod)
            maskM = sb.tile([128, 128], f32, name="maskM")
            nc.sync.dma_start(out=maskM, in_=maskd[:, :])
            # ACT-ring DMA: cos/sin (needed by norm_rope soon after wkv)
            cssn_sb = sb.tile([BS, 2 * D], f32, name="cssn_sb")
            nc.scalar.dma_start(out=cssn_sb, in_=cssn[:, :])
            cos_sb = cssn_sb[:, 0:D]
            sin_sb = cssn_sb[:, D : 2 * D]

            ident = sb.tile([128, 128], f32, name="ident")
            make_identity(nc, ident)

            norm_w_sb = {}
            for flag, name, dram in (
                (apply_qw, "qw_sb", qw if apply_qw else None),
                (apply_kw, "kw_sb", kw if apply_kw else None),
            ):
                if flag:
                    t = sb.tile([BS, D], f32, name=name)
                    src = dram[:, :]
                    bcast = bass.AP(
                        tensor=src.tensor,
                        offset=src.offset,
                        ap=[[0, BS], list(src.ap[-1])],
                    )
                    nc.sync.dma_start(out=t, in_=bcast)
                    norm_w_sb[name] = t

            eps_q = sb.tile([BS, 1], f32, name="eps_q")
            nc.vector.memset(eps_q, float(D) * EPS)
            eps_k = sb.tile([BS, 1], f32, name="eps_k")
            nc.vector.memset(eps_k, EPS)

            qr_sb = sb.tile([BS, G * D], f32, name="qr_sb")
            kr_sb = sb.tile([BS, D], f32, name="kr_sb")
            vbf = sb.tile([BS, D], bf16, name="vbf")
            qT_sb = sb.tile([128, G * BS], bf16, name="qT_sb")
            kTbf = sb.tile([128, BS], bf16, name="kTbf")
            attn_T = sb.tile([128, G * BS], bf16, name="attn_T")

            with tc.tile_pool(name="psA", bufs=1, space="PSUM") as psA:
                q_ps = psA.tile([BS, G * D], f32, name="q_ps")
                kv_ps = psA.tile([BS, 2 * D], f32, name="kv_ps")
                last = KTILES - 1
                # all kv matmuls first: they only need wkv, so the k/v
                # path (and the cache patches) don't wait on the wq DMA
                for t in range(KTILES):
                    nc.tensor.matmul(
                        kv_ps,
                        hid_sb[:, t * BS : (t + 1) * BS],
                        wkv_sb[:, t * 2 * D : (t + 1) * 2 * D],
                        start=(t == 0), stop=(t == last),
                    )
                for t in range(KTILES):
                    nc.tensor.matmul(
                        q_ps,
                        hid_sb[:, t * BS : (t + 1) * BS],
                        wq_sb[:, t * G * D : (t + 1) * G * D],
                        start=(t == 0), stop=(t == last),
                    )
                k_ps = kv_ps[:, 0:D]
                v_ps = kv_ps[:, D : 2 * D]

                def norm_rope(src_ps, dst, head_cnt, is_q):
                    w_sb = norm_w_sb.get("qw_sb" if is_q else "kw_sb")
                    for h in range(head_cnt):
                        xin = src_ps[:, h * D : (h + 1) * D]
                        scratch = smalls.tile([BS, D], f32, name="scratch", tag="scr")
                        ssq = smalls.tile([BS, 1], f32, name="ssq", tag="ssq")
                        nc.scalar.activation(scratch, xin, AF.Square, accum_out=ssq)
                        s = smalls.tile([BS, 1], f32, name="s", tag="s")
                        if is_q:
                            # s = sqrt(sum(q^2) + D*eps): 1/s folds in the
                            # attention scale D**-0.5 on top of the rms norm
                            nc.scalar.activation(s, ssq, AF.Sqrt, bias=eps_q, scale=1.0)
                        else:
                            nc.scalar.activation(s, ssq, AF.Sqrt, bias=eps_k, scale=1.0 / D)
                        inv = smalls.tile([BS, 1], f32, name="inv", tag="inv")
                        nc.vector.reciprocal(inv, s)
                        xn = smalls.tile([BS, D], f32, name="xn", tag="xn")
                        nc.scalar.activation(xn, xin, AF.Copy, scale=inv)
                        if w_sb is not None:
                            nc.vector.tensor_mul(xn, xn, w_sb)
                        rot = smalls.tile([BS, D], f32, name="rot", tag="rot")
                        nc.scalar.mul(rot[:, 0 : D // 2], xn[:, D // 2 : D], -1.0)
                        nc.scalar.copy(rot[:, D // 2 : D], xn[:, 0 : D // 2])
                        t1 = smalls.tile([BS, D], f32, name="t1", tag="t1")
                        nc.vector.tensor_mul(t1, xn, cos_sb)
                        nc.vector.tensor_mul(rot, rot, sin_sb)
                        nc.vector.tensor_add(dst[:, h * D : (h + 1) * D], t1, rot)

                # k/v first: the per-seq cache patches gate on kTbf/vbf
                nc.vector.tensor_copy(vbf, v_ps)
                norm_rope(k_ps, kr_sb, 1, False)

                with tc.tile_pool(name="psT", bufs=2, space="PSUM") as psT:
                    tpk = psT.tile([128, BS], f32, name="tpk", tag="tp")
                    nc.tensor.transpose(tpk, kr_sb, ident[:BS, :BS])
                    nc.vector.tensor_copy(kTbf, tpk)
                    norm_rope(q_ps, qr_sb, G, True)
                    for h in range(G):
                        tp = psT.tile([128, BS], f32, name="tp", tag="tp")
                        nc.tensor.transpose(
                            tp, qr_sb[:, h * D : (h + 1) * D], ident[:BS, :BS]
                        )
                        nc.vector.tensor_copy(qT_sb[:, h * BS : (h + 1) * BS], tp)

            # Wo reuses Wq's SBUF slot; DMA can start once phase A mms done
            wo_sb = sb.tile([128, G * HID], bf16, name="wo_sb", tag="bigw")
            nc.sync.dma_start(out=wo_sb, in_=wo[:, :])

            qT3 = qT_sb.rearrange("p (h c) -> p h c", c=BS)
            attn3 = attn_T.rearrange("p (h c) -> p h c", c=BS)

            with tc.tile_pool(name="psB", bufs=1, space="PSUM") as psB:
                for gi, (g0, seqs) in enumerate(groups):
                    nbg = sum(nb[b] for b in seqs)
                    kt = sb.tile(
                        [128, nbg * BLOCK_SIZE], bf16, name=f"kt{gi}", tag="ktall",
                        bufs=3,
                    )
                    vn = sb.tile(
                        [128, nbg * VW], bf16, name=f"vn{gi}", tag="vnat", bufs=3
                    )
                    nc.sync.dma_start(
                        out=kt, in_=kp[:, g0 * BLOCK_SIZE : (g0 + nbg) * BLOCK_SIZE]
                    )
                    nc.scalar.dma_start(out=vn, in_=vp[:, g0 * VW : (g0 + nbg) * VW])
                    vn3 = vn.rearrange("p (n v) -> p n v", v=VW)
                    # new token's k/v replace the stale cache slot
                    lb = 0
                    for b in seqs:
                        T = int(seq_lens[b])
                        r = (T - 1) % BLOCK_SIZE
                        le = lb + nb[b] - 1  # local index of seq's last block
                        nc.vector.tensor_copy(
                            kt[:, le * BLOCK_SIZE + r : le * BLOCK_SIZE + r + 1],
                            kTbf[:, b : b + 1],
                        )
                        nc.scalar.dma_start(
                            out=vn3[r : r + 1, le, 0:D], in_=vbf[b : b + 1, :]
                        )
                        lb += nb[b]

                    stp = psB.tile([128, 4 * nbg], f32, name=f"stp{gi}", tag="stp",
                                   bufs=2)
                    lb = 0
                    for b in seqs:
                        qTb = qT3[:, :, b]
                        for j in range(nb[b]):
                            jl = lb + j
                            nc.tensor.matmul(
                                stp[:, 4 * jl : 4 * jl + 4],
                                kt[:, jl * BLOCK_SIZE : (jl + 1) * BLOCK_SIZE],
                                qTb,
                                start=True,
                                stop=True,
                            )
                        lb += nb[b]
                    e = sb.tile([128, 4 * nbg], bf16, name=f"e{gi}", tag="e", bufs=2)
                    nc.scalar.activation(e, stp, AF.Exp)
                    lb = 0
                    for b in seqs:
                        tmod = int(seq_lens[b]) % BLOCK_SIZE
                        le = lb + nb[b] - 1
                        if tmod != 0:
                            nc.vector.tensor_scalar_mul(
                                e[:, 4 * le : 4 * le + 4],
                                e[:, 4 * le : 4 * le + 4],
                                maskM[:, tmod : tmod + 1],
                            )
                        lb += nb[b]

                    lb = 0
                    for b in seqs:
                        ot_ps = psB.tile([4, VW], f32, name=f"ot{b}", tag="ot", bufs=2)
                        for j in range(nb[b]):
                            jl = lb + j
                            nc.tensor.matmul(
                                ot_ps,
                                e[:, 4 * jl : 4 * jl + 4],
                                vn3[:, jl, :],
                                start=(j == 0),
                                stop=(j == nb[b] - 1),
                            )
                        lb += nb[b]
                        rec = smalls.tile([4, 1], f32, name=f"rec{b}", tag="rec")
                        nc.vector.reciprocal(rec, ot_ps[:, D : D + 1])
                        o_sb = smalls.tile([4, D], f32, name=f"o{b}", tag="o")
                        nc.vector.tensor_scalar_mul(o_sb, ot_ps[:, 0:D], rec)
                        tp2 = psB.tile([128, 4], f32, name=f"tp2_{b}", tag="tp2",
                                       bufs=2)
                        nc.tensor.transpose(tp2, o_sb, ident[:4, :4])
                        nc.vector.tensor_copy(attn3[:, :, b], tp2)

            with tc.tile_pool(name="psC", bufs=2, space="PSUM") as psC:
                for c in range(HID // 512):
                    oc = psC.tile([BS, 512], f32, name=f"oc{c}", tag="oc")
                    for h in range(G):
                        nc.tensor.matmul(
                            oc,
                            attn_T[:, h * BS : (h + 1) * BS],
                            wo_sb[:, h * HID + c * 512 : h * HID + (c + 1) * 512],
                            start=(h == 0), stop=(h == G - 1),
                        )
                    ocs = sb.tile([BS, 512], f32, name=f"ocs{c}", tag="ocs", bufs=2)
                    nc.vector.tensor_copy(ocs, oc)
                    nc.sync.dma_start(out=outp[:, c * 512 : (c + 1) * 512], in_=ocs)

    nc.compile()
    return nc


def _pack_w(w):
    # [4096, C] -> [128, KTILES*C]; sbuf[p, t*C + c] == w[t*128 + p, c]
    C = w.shape[1]
    return np.ascontiguousarray(
        w.reshape(KTILES, 128, C).transpose(1, 0, 2).reshape(128, KTILES * C)
    ).astype(BF16)


def _prepare(inputs):
    hs = np.ascontiguousarray(np.asarray(inputs["hidden_states"], np.float32)[0])
    Wq = np.asarray(inputs["Wq"], np.float32)
    Wk = np.asarray(inputs["Wk"], np.float32)
    Wv = np.asarray(inputs["Wv"], np.float32)
    Wo = np.asarray(inputs["Wo"], np.float32)
    cos_t = np.asarray(inputs["cos"], np.float32)[0]
    sin_t = np.asarray(inputs["sin"], np.float32)[0]
    qnw = np.asarray(inputs["q_norm_w"], np.float32)
    knw = np.asarray(inputs["k_norm_w"], np.float32)
    key_cache = np.asarray(inputs["key_cache"], np.float32)
    value_cache = np.asarray(inputs["value_cache"], np.float32)
    seq_lens = np.asarray(inputs["seq_lens_k"]).astype(np.int64)
    bt = np.asarray(inputs["block_table"]).astype(np.int64)

    apply_qw = not np.all(qnw == 1.0)
    apply_kw = not np.all(knw == 1.0)

    nb = [(int(t) + BLOCK_SIZE - 1) // BLOCK_SIZE for t in seq_lens]
    nbtot = sum(nb)
    order, _ = _plan(nb)
    blocks = np.concatenate([bt[b, : nb[b]] for b in order])

    hidT = np.ascontiguousarray(
        hs.T.reshape(KTILES, 128, BS).transpose(1, 0, 2).reshape(128, KTILES * BS)
    ).astype(BF16)

    # gather valid blocks once, cast to bf16 once: [nbtot, 128, HKV, D]
    kc_sel = key_cache[blocks].astype(BF16)
    vc_sel = value_cache[blocks].astype(BF16)

    cssn = np.ascontiguousarray(np.concatenate([cos_t, sin_t], axis=1))
    maskd = (np.arange(128)[:, None] < np.arange(128)[None, :]).astype(np.float32)

    in_maps = []
    for i in range(NCORES):
        # K^T: [d, blk, tok] -> [128, nbtot*128]; per-partition contiguous
        kp_i = np.ascontiguousarray(
            kc_sel[:, :, i, :].transpose(2, 0, 1)
        ).reshape(128, nbtot * BLOCK_SIZE)
        # V: [tok, blk, d+pad] -> [128, nbtot*VW]; col D is the ones
        # column whose PV output is the softmax denominator
        vp_i = np.zeros((128, nbtot, VW), BF16)
        vp_i[:, :, 0:D] = vc_sel[:, :, i, :].transpose(1, 0, 2)
        vp_i[:, :, D] = BF16(1.0)
        m = {
            "hidT": hidT,
            "wq": _pack_w(Wq[:, i * G * D : (i + 1) * G * D]),
            "wkv": _pack_w(
                np.concatenate(
                    [Wk[:, i * D : (i + 1) * D], Wv[:, i * D : (i + 1) * D]], axis=1
                )
            ),
            "wo": np.ascontiguousarray(
                Wo[i * G * D : (i + 1) * G * D, :]
                .reshape(G, D, HID)
                .transpose(1, 0, 2)
                .reshape(128, G * HID)
            ).astype(BF16),
            "cssn": cssn,
            "kp": kp_i,
            "vp": vp_i.reshape(128, nbtot * VW),
            "maskd": maskd,
        }
        if apply_qw:
            m["qw"] = np.ascontiguousarray(qnw.reshape(1, D))
        if apply_kw:
            m["kw"] = np.ascontiguousarray(knw.reshape(1, D))
        in_maps.append(m)

    key = (tuple(int(x) for x in seq_lens), apply_qw, apply_kw)
    if key not in _prog_cache:
        _prog_cache[key] = _build_program(seq_lens, apply_qw, apply_kw)
    nc = _prog_cache[key]
    return nc, in_maps


def kernel_with_stats(trace=False, **inputs):
    from concourse.bass_utils import run_bass_kernel_spmd

    nc, in_maps = _prepare(inputs)
    res = run_bass_kernel_spmd(
        nc, in_maps, core_ids=list(range(NCORES)), trace=trace
    )
    out = np.zeros((BS, HID), np.float32)
    for r in res.results:
        out += r["outp"].astype(np.float32)
    return out.reshape(1, BS, HID), res


def kernel(**inputs):
    out, _ = kernel_with_stats(trace=False, **inputs)
    return out


# revision 11
# speedup vs baseline: 1.1136x; 1.1136x over previous
"""Paged GQA decode attention (1 token/seq) on 8 trn2 NeuronCores.

Sharding: tensor-parallel over the 8 KV heads. Core i owns KV head i and
its G=4 query heads: Wq/Wk/Wv column-sharded, Wo row-sharded; each core
produces a partial [BS, HID] output and the host sums the 8 partials.

v3: all matmul operands in bf16 (half the HBM traffic of f32); K packed
on host as [d, tok] and V as [tok, blk, d] so every partition reads one
contiguous run; KV streamed in ~7 group DMAs (first-fit-decreasing over
sequences, small final group to shorten the tail) with triple
buffering; V stream + patches ride the ACT HWDGE ring, K + weights the
SP ring, so both descriptor generators stay busy; one exp ACT per group
over the whole [128, 4*nbg] score tile.

Per-core dataflow (one Bass program, SPMD over cores via per-core inputs):
  A) kv = hidT.T @ Wkv then q = hidT.T @ Wq (PE, 32 k-tiles in PSUM);
     RMSNorm + RoPE on [BS, D] tiles (ACT/DVE), attn scale folded into
     q's inverse-rms; k path runs first so the cache patches unblock.
  B) per group of sequences (blocks packed contiguously):
     one K DMA [128d, nbg*128] + one V DMA [128tok, nbg*130];
     new-token fix: stale cache slot at pos=T-1 gets its KT column /
     V row overwritten with this step's k/v.
     S^T[tok, 4*nbg] accumulated per block (lhsT=K block, rhs=qT_b),
     one exp over the group, per-seq tail mask,
     per-seq O^T[4, 130] += matmul(lhsT=e_blk, rhs=V_blk) with the V
     ones-column giving the softmax denominator.
  C) partial = attn^T.T @ Wo_slice (PE), DMA'd out per 512-col chunk.
"""

import numpy as np
import ml_dtypes

BF16 = ml_dtypes.bfloat16

HID, H, HKV, D = 4096, 32, 8, 128
BS, BLOCKS_PER_SEQ, BLOCK_SIZE = 32, 32, 128
G = H // HKV
EPS = 1e-6
NCORES = 8
KTILES = HID // 128  # contraction tiles for the projections
VW = D + 2  # V row width: 128 values + ones col (denominator) + pad
GMAX = 80  # max cache blocks per streamed group

_prog_cache = {}


def _plan(nb):
    """Pack whole sequences into groups of <= GMAX blocks.

    First-fit decreasing, plus one deliberately tiny final group (the
    sequence closest to 8 blocks) so the post-stream tail is short.
    Returns (pack_order, groups); groups are (start_block, [seq ids])."""
    idx = sorted(range(len(nb)), key=lambda b: -nb[b])
    fin = min(idx, key=lambda b: abs(nb[b] - 8))
    bins = []  # [total_blocks, [seq ids]]
    for b in idx:
        if b == fin:
            continue
        for bin_ in bins:
            if bin_[0] + nb[b] <= GMAX:
                bin_[0] += nb[b]
                bin_[1].append(b)
                break
        else:
            bins.append([nb[b], [b]])
    bins.append([nb[fin], [fin]])
    groups, order, start = [], [], 0
    for tot, seqs in bins:
        groups.append((start, seqs))
        order.extend(seqs)
        start += tot
    return order, groups


def _build_program(seq_lens, apply_qw, apply_kw):
    import concourse.bass as bass
    import concourse.tile as tile
    from concourse import bacc, mybir
    from concourse.masks import make_identity

    f32 = mybir.dt.float32
    bf16 = mybir.dt.bfloat16
    AF = mybir.ActivationFunctionType

    nb = [(int(t) + BLOCK_SIZE - 1) // BLOCK_SIZE for t in seq_lens]
    nbtot = sum(nb)
    _, groups = _plan(nb)

    nc = bacc.Bacc("TRN2", target_bir_lowering=False)
    hidT = nc.dram_tensor("hidT", [128, KTILES * BS], bf16, kind="ExternalInput")
    wq = nc.dram_tensor("wq", [128, KTILES * G * D], bf16, kind="ExternalInput")
    wkv = nc.dram_tensor("wkv", [128, KTILES * 2 * D], bf16, kind="ExternalInput")
    wo = nc.dram_tensor("wo", [128, G * HID], bf16, kind="ExternalInput")
    cssn = nc.dram_tensor("cssn", [BS, 2 * D], f32, kind="ExternalInput")
    kp = nc.dram_tensor("kp", [128, nbtot * BLOCK_SIZE], bf16, kind="ExternalInput")
    vp = nc.dram_tensor("vp", [128, nbtot * VW], bf16, kind="ExternalInput")
    maskd = nc.dram_tensor("maskd", [128, 128], f32, kind="ExternalInput")
    if apply_qw:
        qw = nc.dram_tensor("qw", [1, D], f32, kind="ExternalInput")
    if apply_kw:
        kw = nc.dram_tensor("kw", [1, D], f32, kind="ExternalInput")
    outp = nc.dram_tensor("outp", [BS, HID], f32, kind="ExternalOutput")

    with tile.TileContext(nc) as tc:
        with tc.tile_pool(name="sb", bufs=1) as sb, tc.tile_pool(
            name="smalls", bufs=4
        ) as smalls:
            # SP-ring DMAs first: the phase-A critical path
            hid_sb = sb.tile([128, KTILES * BS], bf16, name="hid_sb")
            nc.sync.dma_start(out=hid_sb, in_=hidT[:, :])
            wkv_sb = sb.tile([128, KTILES * 2 * D], bf16, name="wkv_sb")
            nc.sync.dma_start(out=wkv_sb, in_=wkv[:, :])
            wq_sb = sb.tile([128, KTILES * G * D], bf16, name="wq_sb", tag="bigw")
            nc.sync.dma_start(out=wq_sb, in_=wq[:, :])
            # maskM[p, c] = 1.0 if p < c else 0.0 (tail masks for all Tmod)
            maskM = sb.tile([128, 128], f32, name="maskM")
            nc.sync.dma_start(out=maskM, in_=maskd[:, :])
            # ACT-ring DMA: cos/sin (needed by norm_rope soon after wkv);
            # everything else stays on the SP ring — streaming DMAs on the
            # ACT ring serialize with exp (engine head-of-line blocking)
            cssn_sb = sb.tile([BS, 2 * D], f32, name="cssn_sb")
            nc.scalar.dma_start(out=cssn_sb, in_=cssn[:, :])
            cos_sb = cssn_sb[:, 0:D]
            sin_sb = cssn_sb[:, D : 2 * D]

            ident = sb.tile([128, 128], f32, name="ident")
            make_identity(nc, ident)

            norm_w_sb = {}
            for flag, name, dram in (
                (apply_qw, "qw_sb", qw if apply_qw else None),
                (apply_kw, "kw_sb", kw if apply_kw else None),
            ):
                if flag:
                    t = sb.tile([BS, D], f32, name=name)
                    src = dram[:, :]
                    bcast = bass.AP(
                        tensor=src.tensor,
                        offset=src.offset,
                        ap=[[0, BS], list(src.ap[-1])],
                    )
                    nc.sync.dma_start(out=t, in_=bcast)
                    norm_w_sb[name] = t

            eps_q = sb.tile([BS, 1], f32, name="eps_q")
            nc.vector.memset(eps_q, float(D) * EPS)
            eps_k = sb.tile([BS, 1], f32, name="eps_k")
            nc.vector.memset(eps_k, EPS)

            qr_sb = sb.tile([BS, G * D], f32, name="qr_sb")
            kr_sb = sb.tile([BS, D], f32, name="kr_sb")
            vbf = sb.tile([BS, D], bf16, name="vbf")
            qT_sb = sb.tile([128, G * BS], bf16, name="qT_sb")
            kTbf = sb.tile([128, BS], bf16, name="kTbf")
            attn_T = sb.tile([128, G * BS], bf16, name="attn_T")

            with tc.tile_pool(name="psA", bufs=1, space="PSUM") as psA:
                q_ps = psA.tile([BS, G * D], f32, name="q_ps")
                kv_ps = psA.tile([BS, 2 * D], f32, name="kv_ps")
                last = KTILES - 1
                # all kv matmuls first: they only need wkv, so the k/v
                # path (and the cache patches) don't wait on the wq DMA
                for t in range(KTILES):
                    nc.tensor.matmul(
                        kv_ps,
                        hid_sb[:, t * BS : (t + 1) * BS],
                        wkv_sb[:, t * 2 * D : (t + 1) * 2 * D],
                        start=(t == 0), stop=(t == last),
                    )
                for t in range(KTILES):
                    nc.tensor.matmul(
                        q_ps,
                        hid_sb[:, t * BS : (t + 1) * BS],
                        wq_sb[:, t * G * D : (t + 1) * G * D],
                        start=(t == 0), stop=(t == last),
                    )
                k_ps = kv_ps[:, 0:D]
                v_ps = kv_ps[:, D : 2 * D]

                def norm_rope(src_ps, dst, head_cnt, is_q):
                    w_sb = norm_w_sb.get("qw_sb" if is_q else "kw_sb")
                    for h in range(head_cnt):
                        xin = src_ps[:, h * D : (h + 1) * D]
                        scratch = smalls.tile([BS, D], f32, name="scratch", tag="scr")
                        ssq = smalls.tile([BS, 1], f32, name="ssq", tag="ssq")
                        nc.scalar.activation(scratch, xin, AF.Square, accum_out=ssq)
                        s = smalls.tile([BS, 1], f32, name="s", tag="s")
                        if is_q:
                            # s = sqrt(sum(q^2) + D*eps): 1/s folds in the
                            # attention scale D**-0.5 on top of the rms norm
                            nc.scalar.activation(s, ssq, AF.Sqrt, bias=eps_q, scale=1.0)
                        else:
                            nc.scalar.activation(s, ssq, AF.Sqrt, bias=eps_k, scale=1.0 / D)
                        inv = smalls.tile([BS, 1], f32, name="inv", tag="inv")
                        nc.vector.reciprocal(inv, s)
                        xn = smalls.tile([BS, D], f32, name="xn", tag="xn")
                        nc.scalar.activation(xn, xin, AF.Copy, scale=inv)
                        if w_sb is not None:
                            nc.vector.tensor_mul(xn, xn, w_sb)
                        rot = smalls.tile([BS, D], f32, name="rot", tag="rot")
                        nc.scalar.mul(rot[:, 0 : D // 2], xn[:, D // 2 : D], -1.0)
                        nc.scalar.copy(rot[:, D // 2 : D], xn[:, 0 : D // 2])
                        t1 = smalls.tile([BS, D], f32, name="t1", tag="t1")
                        nc.vector.tensor_mul(t1, xn, cos_sb)
                        nc.vector.tensor_mul(rot, rot, sin_sb)
                        nc.vector.tensor_add(dst[:, h * D : (h + 1) * D], t1, rot)

                # k/v first: the per-seq cache patches gate on kTbf/vbf
                nc.vector.tensor_copy(vbf, v_ps)
                norm_rope(k_ps, kr_sb, 1, False)

                with tc.tile_pool(name="psT", bufs=2, space="PSUM") as psT:
                    tpk = psT.tile([128, BS], f32, name="tpk", tag="tp")
                    nc.tensor.transpose(tpk, kr_sb, ident[:BS, :BS])
                    nc.vector.tensor_copy(kTbf, tpk)
                    norm_rope(q_ps, qr_sb, G, True)
                    for h in range(G):
                        tp = psT.tile([128, BS], f32, name="tp", tag="tp")
                        nc.tensor.transpose(
                            tp, qr_sb[:, h * D : (h + 1) * D], ident[:BS, :BS]
                        )
                        nc.vector.tensor_copy(qT_sb[:, h * BS : (h + 1) * BS], tp)

            # Wo reuses Wq's SBUF slot; DMA can start once phase A mms done
            wo_sb = sb.tile([128, G * HID], bf16, name="wo_sb", tag="bigw")
            nc.sync.dma_start(out=wo_sb, in_=wo[:, :])

            qT3 = qT_sb.rearrange("p (h c) -> p h c", c=BS)
            attn3 = attn_T.rearrange("p (h c) -> p h c", c=BS)

            with tc.tile_pool(name="psB", bufs=1, space="PSUM") as psB:
                for gi, (g0, seqs) in enumerate(groups):
                    nbg = sum(nb[b] for b in seqs)
                    kt = sb.tile(
                        [128, nbg * BLOCK_SIZE], bf16, name=f"kt{gi}", tag="ktall",
                        bufs=3,
                    )
                    vn = sb.tile(
                        [128, nbg * VW], bf16, name=f"vn{gi}", tag="vnat", bufs=3
                    )
                    nc.sync.dma_start(
                        out=kt, in_=kp[:, g0 * BLOCK_SIZE : (g0 + nbg) * BLOCK_SIZE]
                    )
                    nc.sync.dma_start(out=vn, in_=vp[:, g0 * VW : (g0 + nbg) * VW])
                    vn3 = vn.rearrange("p (n v) -> p n v", v=VW)
                    # new token's k/v replace the stale cache slot
                    lb = 0
                    for b in seqs:
                        T = int(seq_lens[b])
                        r = (T - 1) % BLOCK_SIZE
                        le = lb + nb[b] - 1  # local index of seq's last block
                        nc.vector.tensor_copy(
                            kt[:, le * BLOCK_SIZE + r : le * BLOCK_SIZE + r + 1],
                            kTbf[:, b : b + 1],
                        )
                        nc.sync.dma_start(
                            out=vn3[r : r + 1, le, 0:D], in_=vbf[b : b + 1, :]
                        )
                        lb += nb[b]

                    stp = psB.tile([128, 4 * nbg], f32, name=f"stp{gi}", tag="stp",
                                   bufs=2)
                    lb = 0
                    for b in seqs:
                        qTb = qT3[:, :, b]
                        for j in range(nb[b]):
                            jl = lb + j
                            nc.tensor.matmul(
                                stp[:, 4 * jl : 4 * jl + 4],
                                kt[:, jl * BLOCK_SIZE : (jl + 1) * BLOCK_SIZE],
                                qTb,
                                start=True,
                                stop=True,
                            )
                        lb += nb[b]
                    e = sb.tile([128, 4 * nbg], bf16, name=f"e{gi}", tag="e", bufs=2)
                    nc.scalar.activation(e, stp, AF.Exp)
                    lb = 0
                    for b in seqs:
                        tmod = int(seq_lens[b]) % BLOCK_SIZE
                        le = lb + nb[b] - 1
                        if tmod != 0:
                            nc.vector.tensor_scalar_mul(
                                e[:, 4 * le : 4 * le + 4],
                                e[:, 4 * le : 4 * le + 4],
                                maskM[:, tmod : tmod + 1],
                            )
                        lb += nb[b]

                    lb = 0
                    for b in seqs:
                        ot_ps = psB.tile([4, VW], f32, name=f"ot{b}", tag="ot", bufs=2)
                        for j in range(nb[b]):
                            jl = lb + j
                            nc.tensor.matmul(
                                ot_ps,
                                e[:, 4 * jl : 4 * jl + 4],
                                vn3[:, jl, :],
                                start=(j == 0),
                                stop=(j == nb[b] - 1),
                            )
                        lb += nb[b]
                        rec = smalls.tile([4, 1], f32, name=f"rec{b}", tag="rec")
                        nc.vector.reciprocal(rec, ot_ps[:, D : D + 1])
                        o_sb = smalls.tile([4, D], f32, name=f"o{b}", tag="o")
                        nc.vector.tensor_scalar_mul(o_sb, ot_ps[:, 0:D], rec)
                        tp2 = psB.tile([128, 4], f32, name=f"tp2_{b}", tag="tp2",
                                       bufs=2)
                        nc.tensor.transpose(tp2, o_sb, ident[:4, :4])
                        nc.vector.tensor_copy(attn3[:, :, b], tp2)

            with tc.tile_pool(name="psC", bufs=4, space="PSUM") as psC:
                for c in range(HID // 512):
                    oc = psC.tile([BS, 512], f32, name=f"oc{c}", tag="oc")
                    for h in range(G):
                        nc.tensor.matmul(
                            oc,
                            attn_T[:, h * BS : (h + 1) * BS],
                            wo_sb[:, h * HID + c * 512 : h * HID + (c + 1) * 512],
                            start=(h == 0), stop=(h == G - 1),
                        )
                    ocs = sb.tile([BS, 512], f32, name=f"ocs{c}", tag="ocs", bufs=4)
                    nc.vector.tensor_copy(ocs, oc)
                    nc.sync.dma_start(out=outp[:, c * 512 : (c + 1) * 512], in_=ocs)

    nc.compile()
    return nc


def _pack_w(w):
    # [4096, C] -> [128, KTILES*C]; sbuf[p, t*C + c] == w[t*128 + p, c]
    C = w.shape[1]
    return np.ascontiguousarray(
        w.reshape(KTILES, 128, C).transpose(1, 0, 2).reshape(128, KTILES * C)
    ).astype(BF16)


def _prepare(inputs):
    hs = np.ascontiguousarray(np.asarray(inputs["hidden_states"], np.float32)[0])
    Wq = np.asarray(inputs["Wq"], np.float32)
    Wk = np.asarray(inputs["Wk"], np.float32)
    Wv = np.asarray(inputs["Wv"], np.float32)
    Wo = np.asarray(inputs["Wo"], np.float32)
    cos_t = np.asarray(inputs["cos"], np.float32)[0]
    sin_t = np.asarray(inputs["sin"], np.float32)[0]
    qnw = np.asarray(inputs["q_norm_w"], np.float32)
    knw = np.asarray(inputs["k_norm_w"], np.float32)
    key_cache = np.asarray(inputs["key_cache"], np.float32)
    value_cache = np.asarray(inputs["value_cache"], np.float32)
    seq_lens = np.asarray(inputs["seq_lens_k"]).astype(np.int64)
    bt = np.asarray(inputs["block_table"]).astype(np.int64)

    apply_qw = not np.all(qnw == 1.0)
    apply_kw = not np.all(knw == 1.0)

    nb = [(int(t) + BLOCK_SIZE - 1) // BLOCK_SIZE for t in seq_lens]
    nbtot = sum(nb)
    order, _ = _plan(nb)
    blocks = np.concatenate([bt[b, : nb[b]] for b in order])

    hidT = np.ascontiguousarray(
        hs.T.reshape(KTILES, 128, BS).transpose(1, 0, 2).reshape(128, KTILES * BS)
    ).astype(BF16)

    # gather valid blocks once, cast to bf16 once: [nbtot, 128, HKV, D]
    kc_sel = key_cache[blocks].astype(BF16)
    vc_sel = value_cache[blocks].astype(BF16)

    cssn = np.ascontiguousarray(np.concatenate([cos_t, sin_t], axis=1))
    maskd = (np.arange(128)[:, None] < np.arange(128)[None, :]).astype(np.float32)

    in_maps = []
    for i in range(NCORES):
        # K^T: [d, blk, tok] -> [128, nbtot*128]; per-partition contiguous
        kp_i = np.ascontiguousarray(
            kc_sel[:, :, i, :].transpose(2, 0, 1)
        ).reshape(128, nbtot * BLOCK_SIZE)
        # V: [tok, blk, d+pad] -> [128, nbtot*VW]; col D is the ones
        # column whose PV output is the softmax denominator
        vp_i = np.zeros((128, nbtot, VW), BF16)
        vp_i[:, :, 0:D] = vc_sel[:, :, i, :].transpose(1, 0, 2)
        vp_i[:, :, D] = BF16(1.0)
        m = {
            "hidT": hidT,
            "wq": _pack_w(Wq[:, i * G * D : (i + 1) * G * D]),
            "wkv": _pack_w(
                np.concatenate(
                    [Wk[:, i * D : (i + 1) * D], Wv[:, i * D : (i + 1) * D]], axis=1
                )
            ),
            "wo": np.ascontiguousarray(
                Wo[i * G * D : (i + 1) * G * D, :]
                .reshape(G, D, HID)
                .transpose(1, 0, 2)
                .reshape(128, G * HID)
            ).astype(BF16),
            "cssn": cssn,
            "kp": kp_i,
            "vp": vp_i.reshape(128, nbtot * VW),
            "maskd": maskd,
        }
        if apply_qw:
            m["qw"] = np.ascontiguousarray(qnw.reshape(1, D))
        if apply_kw:
            m["kw"] = np.ascontiguousarray(knw.reshape(1, D))
        in_maps.append(m)

    key = (tuple(int(x) for x in seq_lens), apply_qw, apply_kw)
    if key not in _prog_cache:
        _prog_cache[key] = _build_program(seq_lens, apply_qw, apply_kw)
    nc = _prog_cache[key]
    return nc, in_maps


def kernel_with_stats(trace=False, **inputs):
    from concourse.bass_utils import run_bass_kernel_spmd

    nc, in_maps = _prepare(inputs)
    res = run_bass_kernel_spmd(
        nc, in_maps, core_ids=list(range(NCORES)), trace=trace
    )
    out = np.zeros((BS, HID), np.float32)
    for r in res.results:
        out += r["outp"].astype(np.float32)
    return out.reshape(1, BS, HID), res


def kernel(**inputs):
    out, _ = kernel_with_stats(trace=False, **inputs)
    return out


# revision 18
# speedup vs baseline: 1.1379x; 1.0218x over previous
"""Paged GQA decode attention (1 token/seq) on 8 trn2 NeuronCores.

Sharding: tensor-parallel over the 8 KV heads. Core i owns KV head i and
its G=4 query heads: Wq/Wk/Wv column-sharded, Wo row-sharded; each core
produces a partial [BS, HID] output and the host sums the 8 partials.

v3: all matmul operands in bf16 (half the HBM traffic of f32); K packed
on host as [d, tok] and V as [tok, blk, d] so every partition reads one
contiguous run; KV streamed in ~7 group DMAs (first-fit-decreasing over
sequences, small final group to shorten the tail) with triple
buffering; V stream + patches ride the ACT HWDGE ring, K + weights the
SP ring, so both descriptor generators stay busy; one exp ACT per group
over the whole [128, 4*nbg] score tile.

Per-core dataflow (one Bass program, SPMD over cores via per-core inputs):
  A) kv = hidT.T @ Wkv then q = hidT.T @ Wq (PE, 32 k-tiles in PSUM);
     RMSNorm + RoPE on [BS, D] tiles (ACT/DVE), attn scale folded into
     q's inverse-rms; k path runs first so the cache patches unblock.
  B) per group of sequences (blocks packed contiguously):
     one K DMA [128d, nbg*128] + one V DMA [128tok, nbg*130];
     new-token fix: stale cache slot at pos=T-1 gets its KT column /
     V row overwritten with this step's k/v.
     S^T[tok, 4*nbg] accumulated per block (lhsT=K block, rhs=qT_b),
     one exp over the group, per-seq tail mask,
     per-seq O^T[4, 130] += matmul(lhsT=e_blk, rhs=V_blk) with the V
     ones-column giving the softmax denominator.
  C) partial = attn^T.T @ Wo_slice (PE), DMA'd out per 512-col chunk.
"""

import numpy as np
import ml_dtypes

BF16 = ml_dtypes.bfloat16

HID, H, HKV, D = 4096, 32, 8, 128
BS, BLOCKS_PER_SEQ, BLOCK_SIZE = 32, 32, 128
G = H // HKV
EPS = 1e-6
NCORES = 8
KTILES = HID // 128  # contraction tiles for the projections
VW = D + 2  # V row width: 128 values + ones col (denominator) + pad
GMAX = 64  # max cache blocks per streamed group
EXPB = 16  # blocks per exp sub-chunk (pipelines exp/PV under the S matmuls)
WCHUNK = 8  # k-tiles per weight DMA chunk (phase A starts on first chunk)

_prog_cache = {}


def _plan(nb):
    """Pack whole sequences into groups of <= GMAX blocks.

    First-fit decreasing, plus one deliberately tiny final group (the
    sequence closest to 8 blocks) so the post-stream tail is short.
    Returns (pack_order, groups); groups are (start_block, [seq ids])."""
    idx = sorted(range(len(nb)), key=lambda b: -nb[b])
    fin = min(idx, key=lambda b: abs(nb[b] - 8))
    bins = []  # [total_blocks, [seq ids]]
    for b in idx:
        if b == fin:
            continue
        for bin_ in bins:
            if bin_[0] + nb[b] <= GMAX:
                bin_[0] += nb[b]
                bin_[1].append(b)
                break
        else:
            bins.append([nb[b], [b]])
    bins.append([nb[fin], [fin]])
    groups, order, start = [], [], 0
    for tot, seqs in bins:
        groups.append((start, seqs))
        order.extend(seqs)
        start += tot
    return order, groups


def _build_program(seq_lens, apply_qw, apply_kw):
    import concourse.bass as bass
    import concourse.tile as tile
    from concourse import bacc, mybir

    f32 = mybir.dt.float32
    bf16 = mybir.dt.bfloat16
    AF = mybir.ActivationFunctionType

    nb = [(int(t) + BLOCK_SIZE - 1) // BLOCK_SIZE for t in seq_lens]
    nbtot = sum(nb)
    _, groups = _plan(nb)

    nc = bacc.Bacc("TRN2", target_bir_lowering=False)
    hidT = nc.dram_tensor("hidT", [128, KTILES * BS], bf16, kind="ExternalInput")
    wq = nc.dram_tensor("wq", [128, KTILES * G * D], bf16, kind="ExternalInput")
    wkv = nc.dram_tensor("wkv", [128, KTILES * 2 * D], bf16, kind="ExternalInput")
    wo = nc.dram_tensor("wo", [128, G * HID], bf16, kind="ExternalInput")
    cssn = nc.dram_tensor("cssn", [BS, 2 * D], f32, kind="ExternalInput")
    kp = nc.dram_tensor("kp", [128, nbtot * BLOCK_SIZE], bf16, kind="ExternalInput")
    vp = nc.dram_tensor("vp", [128, nbtot * VW], bf16, kind="ExternalInput")
    maskd = nc.dram_tensor("maskd", [128, 128], f32, kind="ExternalInput")
    identd = nc.dram_tensor("identd", [BS, BS], f32, kind="ExternalInput")
    if apply_qw:
        qw = nc.dram_tensor("qw", [1, D], f32, kind="ExternalInput")
    if apply_kw:
        kw = nc.dram_tensor("kw", [1, D], f32, kind="ExternalInput")
    outp = nc.dram_tensor("outp", [BS, HID], f32, kind="ExternalOutput")

    with tile.TileContext(nc) as tc:
        with tc.tile_pool(name="sb", bufs=1) as sb, tc.tile_pool(
            name="smalls", bufs=4
        ) as smalls:
            # SP-ring DMAs first: the phase-A critical path; wkv/wq are
            # chunked so the projection matmuls start on the first chunk
            hid_sb = sb.tile([128, KTILES * BS], bf16, name="hid_sb")
            nc.sync.dma_start(out=hid_sb, in_=hidT[:, :])
            wkv_sb = sb.tile([128, KTILES * 2 * D], bf16, name="wkv_sb")
            for c0 in range(0, KTILES, WCHUNK):
                nc.sync.dma_start(
                    out=wkv_sb[:, c0 * 2 * D : (c0 + WCHUNK) * 2 * D],
                    in_=wkv[:, c0 * 2 * D : (c0 + WCHUNK) * 2 * D],
                )
            wq_sb = sb.tile([128, KTILES * G * D], bf16, name="wq_sb", tag="bigw")
            for c0 in range(0, KTILES, WCHUNK):
                nc.sync.dma_start(
                    out=wq_sb[:, c0 * G * D : (c0 + WCHUNK) * G * D],
                    in_=wq[:, c0 * G * D : (c0 + WCHUNK) * G * D],
                )
            # maskM[p, c] = 1.0 if p < c else 0.0 (tail masks for all Tmod)
            maskM = sb.tile([128, 128], f32, name="maskM")
            nc.sync.dma_start(out=maskM, in_=maskd[:, :])
            ident = sb.tile([BS, BS], f32, name="ident")
            nc.sync.dma_start(out=ident, in_=identd[:, :])
            # ACT-ring DMA: cos/sin (needed by norm_rope soon after wkv);
            # everything else stays on the SP ring — streaming DMAs on the
            # ACT ring serialize with exp (engine head-of-line blocking)
            cssn_sb = sb.tile([BS, 2 * D], f32, name="cssn_sb")
            nc.scalar.dma_start(out=cssn_sb, in_=cssn[:, :])
            cos_sb = cssn_sb[:, 0:D]
            sin_sb = cssn_sb[:, D : 2 * D]

            norm_w_sb = {}
            for flag, name, dram in (
                (apply_qw, "qw_sb", qw if apply_qw else None),
                (apply_kw, "kw_sb", kw if apply_kw else None),
            ):
                if flag:
                    t = sb.tile([BS, D], f32, name=name)
                    src = dram[:, :]
                    bcast = bass.AP(
                        tensor=src.tensor,
                        offset=src.offset,
                        ap=[[0, BS], list(src.ap[-1])],
                    )
                    nc.sync.dma_start(out=t, in_=bcast)
                    norm_w_sb[name] = t

            eps_q = sb.tile([BS, 1], f32, name="eps_q")
            nc.vector.memset(eps_q, float(D) * EPS)
            eps_k = sb.tile([BS, 1], f32, name="eps_k")
            nc.vector.memset(eps_k, EPS)

            qr_sb = sb.tile([BS, G * D], f32, name="qr_sb")
            kr_sb = sb.tile([BS, D], f32, name="kr_sb")
            vbf = sb.tile([BS, D], bf16, name="vbf")
            qT_sb = sb.tile([128, G * BS], bf16, name="qT_sb")
            kTbf = sb.tile([128, BS], bf16, name="kTbf")
            attn_T = sb.tile([128, G * BS], bf16, name="attn_T")

            with tc.tile_pool(name="psA", bufs=1, space="PSUM") as psA:
                q_ps = psA.tile([BS, G * D], f32, name="q_ps")
                kv_ps = psA.tile([BS, 2 * D], f32, name="kv_ps")
                last = KTILES - 1
                # all kv matmuls first: they only need wkv, so the k/v
                # path (and the cache patches) don't wait on the wq DMA
                for t in range(KTILES):
                    nc.tensor.matmul(
                        kv_ps,
                        hid_sb[:, t * BS : (t + 1) * BS],
                        wkv_sb[:, t * 2 * D : (t + 1) * 2 * D],
                        start=(t == 0), stop=(t == last),
                    )
                for t in range(KTILES):
                    nc.tensor.matmul(
                        q_ps,
                        hid_sb[:, t * BS : (t + 1) * BS],
                        wq_sb[:, t * G * D : (t + 1) * G * D],
                        start=(t == 0), stop=(t == last),
                    )
                k_ps = kv_ps[:, 0:D]
                v_ps = kv_ps[:, D : 2 * D]

                def norm_rope(src_ps, dst, head_cnt, is_q):
                    w_sb = norm_w_sb.get("qw_sb" if is_q else "kw_sb")
                    for h in range(head_cnt):
                        xin = src_ps[:, h * D : (h + 1) * D]
                        scratch = smalls.tile([BS, D], f32, name="scratch", tag="scr")
                        ssq = smalls.tile([BS, 1], f32, name="ssq", tag="ssq")
                        nc.scalar.activation(scratch, xin, AF.Square, accum_out=ssq)
                        s = smalls.tile([BS, 1], f32, name="s", tag="s")
                        if is_q:
                            # s = sqrt(sum(q^2) + D*eps): 1/s folds in the
                            # attention scale D**-0.5 on top of the rms norm
                            nc.scalar.activation(s, ssq, AF.Sqrt, bias=eps_q, scale=1.0)
                        else:
                            nc.scalar.activation(s, ssq, AF.Sqrt, bias=eps_k, scale=1.0 / D)
                        inv = smalls.tile([BS, 1], f32, name="inv", tag="inv")
                        nc.vector.reciprocal(inv, s)
                        xn = smalls.tile([BS, D], f32, name="xn", tag="xn")
                        nc.scalar.activation(xn, xin, AF.Copy, scale=inv)
                        if w_sb is not None:
                            nc.vector.tensor_mul(xn, xn, w_sb)
                        rot = smalls.tile([BS, D], f32, name="rot", tag="rot")
                        nc.scalar.mul(rot[:, 0 : D // 2], xn[:, D // 2 : D], -1.0)
                        nc.scalar.copy(rot[:, D // 2 : D], xn[:, 0 : D // 2])
                        t1 = smalls.tile([BS, D], f32, name="t1", tag="t1")
                        nc.vector.tensor_mul(t1, xn, cos_sb)
                        nc.vector.tensor_mul(rot, rot, sin_sb)
                        nc.vector.tensor_add(dst[:, h * D : (h + 1) * D], t1, rot)

                # k/v first: the per-seq cache patches gate on kTbf/vbf
                nc.vector.tensor_copy(vbf, v_ps)
                norm_rope(k_ps, kr_sb, 1, False)

                with tc.tile_pool(name="psT", bufs=2, space="PSUM") as psT:
                    tpk = psT.tile([128, BS], f32, name="tpk", tag="tp")
                    nc.tensor.transpose(tpk, kr_sb, ident[:BS, :BS])
                    nc.vector.tensor_copy(kTbf, tpk)
                    norm_rope(q_ps, qr_sb, G, True)
                    for h in range(G):
                        tp = psT.tile([128, BS], f32, name="tp", tag="tp")
                        nc.tensor.transpose(
                            tp, qr_sb[:, h * D : (h + 1) * D], ident[:BS, :BS]
                        )
                        nc.vector.tensor_copy(qT_sb[:, h * BS : (h + 1) * BS], tp)

            # Wo reuses Wq's SBUF slot; DMA can start once phase A mms done
            wo_sb = sb.tile([128, G * HID], bf16, name="wo_sb", tag="bigw")
            nc.sync.dma_start(out=wo_sb, in_=wo[:, :])

            qT3 = qT_sb.rearrange("p (h c) -> p h c", c=BS)
            attn3 = attn_T.rearrange("p (h c) -> p h c", c=BS)

            with tc.tile_pool(name="psB", bufs=1, space="PSUM") as psB:
                for gi, (g0, seqs) in enumerate(groups):
                    nbg = sum(nb[b] for b in seqs)
                    kt = sb.tile(
                        [128, nbg * BLOCK_SIZE], bf16, name=f"kt{gi}", tag="ktall",
                        bufs=4,
                    )
                    vn = sb.tile(
                        [128, nbg * VW], bf16, name=f"vn{gi}", tag="vnat", bufs=4
                    )
                    nc.sync.dma_start(
                        out=kt, in_=kp[:, g0 * BLOCK_SIZE : (g0 + nbg) * BLOCK_SIZE]
                    )
                    nc.sync.dma_start(out=vn, in_=vp[:, g0 * VW : (g0 + nbg) * VW])
                    vn3 = vn.rearrange("p (n v) -> p n v", v=VW)
                    # new token's k/v replace the stale cache slot
                    lb = 0
                    for b in seqs:
                        T = int(seq_lens[b])
                        r = (T - 1) % BLOCK_SIZE
                        le = lb + nb[b] - 1  # local index of seq's last block
                        nc.vector.tensor_copy(
                            kt[:, le * BLOCK_SIZE + r : le * BLOCK_SIZE + r + 1],
                            kTbf[:, b : b + 1],
                        )
                        nc.sync.dma_start(
                            out=vn3[r : r + 1, le, 0:D], in_=vbf[b : b + 1, :]
                        )
                        lb += nb[b]

                    stp = psB.tile([128, 4 * nbg], f32, name=f"stp{gi}", tag="stp",
                                   bufs=2)
                    lb = 0
                    for b in seqs:
                        qTb = qT3[:, :, b]
                        for j in range(nb[b]):
                            jl = lb + j
                            nc.tensor.matmul(
                                stp[:, 4 * jl : 4 * jl + 4],
                                kt[:, jl * BLOCK_SIZE : (jl + 1) * BLOCK_SIZE],
                                qTb,
                                start=True,
                                stop=True,
                            )
                        lb += nb[b]
                    e = sb.tile([128, 4 * nbg], bf16, name=f"e{gi}", tag="e", bufs=2)
                    # exp in sub-chunks so PV starts before all S mms finish
                    for c0 in range(0, nbg, EXPB):
                        c1 = min(c0 + EXPB, nbg)
                        nc.scalar.activation(
                            e[:, 4 * c0 : 4 * c1], stp[:, 4 * c0 : 4 * c1], AF.Exp
                        )
                    lb = 0
                    for b in seqs:
                        tmod = int(seq_lens[b]) % BLOCK_SIZE
                        le = lb + nb[b] - 1
                        if tmod != 0:
                            nc.vector.tensor_scalar_mul(
                                e[:, 4 * le : 4 * le + 4],
                                e[:, 4 * le : 4 * le + 4],
                                maskM[:, tmod : tmod + 1],
                            )
                        lb += nb[b]

                    lb = 0
                    for b in seqs:
                        ot_ps = psB.tile([4, VW], f32, name=f"ot{b}", tag="ot", bufs=2)
                        for j in range(nb[b]):
                            jl = lb + j
                            nc.tensor.matmul(
                                ot_ps,
                                e[:, 4 * jl : 4 * jl + 4],
                                vn3[:, jl, :],
                                start=(j == 0),
                                stop=(j == nb[b] - 1),
                            )
                        lb += nb[b]
                        rec = smalls.tile([4, 1], f32, name=f"rec{b}", tag="rec")
                        nc.vector.reciprocal(rec, ot_ps[:, D : D + 1])
                        o_sb = smalls.tile([4, D], f32, name=f"o{b}", tag="o")
                        nc.vector.tensor_scalar_mul(o_sb, ot_ps[:, 0:D], rec)
                        tp2 = psB.tile([128, 4], f32, name=f"tp2_{b}", tag="tp2",
                                       bufs=2)
                        nc.tensor.transpose(tp2, o_sb, ident[:4, :4])
                        nc.vector.tensor_copy(attn3[:, :, b], tp2)

            with tc.tile_pool(name="psC", bufs=4, space="PSUM") as psC:
                for c in range(HID // 512):
                    oc = psC.tile([BS, 512], f32, name=f"oc{c}", tag="oc")
                    for h in range(G):
                        nc.tensor.matmul(
                            oc,
                            attn_T[:, h * BS : (h + 1) * BS],
                            wo_sb[:, h * HID + c * 512 : h * HID + (c + 1) * 512],
                            start=(h == 0), stop=(h == G - 1),
                        )
                    ocs = sb.tile([BS, 512], f32, name=f"ocs{c}", tag="ocs", bufs=4)
                    nc.vector.tensor_copy(ocs, oc)
                    nc.sync.dma_start(out=outp[:, c * 512 : (c + 1) * 512], in_=ocs)

    nc.compile()
    return nc


def _pack_w(w):
    # [4096, C] -> [128, KTILES*C]; sbuf[p, t*C + c] == w[t*128 + p, c]
    C = w.shape[1]
    return np.ascontiguousarray(
        w.reshape(KTILES, 128, C).transpose(1, 0, 2).reshape(128, KTILES * C)
    ).astype(BF16)


def _prepare(inputs):
    hs = np.ascontiguousarray(np.asarray(inputs["hidden_states"], np.float32)[0])
    Wq = np.asarray(inputs["Wq"], np.float32)
    Wk = np.asarray(inputs["Wk"], np.float32)
    Wv = np.asarray(inputs["Wv"], np.float32)
    Wo = np.asarray(inputs["Wo"], np.float32)
    cos_t = np.asarray(inputs["cos"], np.float32)[0]
    sin_t = np.asarray(inputs["sin"], np.float32)[0]
    qnw = np.asarray(inputs["q_norm_w"], np.float32)
    knw = np.asarray(inputs["k_norm_w"], np.float32)
    key_cache = np.asarray(inputs["key_cache"], np.float32)
    value_cache = np.asarray(inputs["value_cache"], np.float32)
    seq_lens = np.asarray(inputs["seq_lens_k"]).astype(np.int64)
    bt = np.asarray(inputs["block_table"]).astype(np.int64)

    apply_qw = not np.all(qnw == 1.0)
    apply_kw = not np.all(knw == 1.0)

    nb = [(int(t) + BLOCK_SIZE - 1) // BLOCK_SIZE for t in seq_lens]
    nbtot = sum(nb)
    order, _ = _plan(nb)
    blocks = np.concatenate([bt[b, : nb[b]] for b in order])

    hidT = np.ascontiguousarray(
        hs.T.reshape(KTILES, 128, BS).transpose(1, 0, 2).reshape(128, KTILES * BS)
    ).astype(BF16)

    # gather valid blocks once, cast to bf16 once: [nbtot, 128, HKV, D]
    kc_sel = key_cache[blocks].astype(BF16)
    vc_sel = value_cache[blocks].astype(BF16)

    cssn = np.ascontiguousarray(np.concatenate([cos_t, sin_t], axis=1))
    maskd = (np.arange(128)[:, None] < np.arange(128)[None, :]).astype(np.float32)

    in_maps = []
    for i in range(NCORES):
        # K^T: [d, blk, tok] -> [128, nbtot*128]; per-partition contiguous
        kp_i = np.ascontiguousarray(
            kc_sel[:, :, i, :].transpose(2, 0, 1)
        ).reshape(128, nbtot * BLOCK_SIZE)
        # V: [tok, blk, d+pad] -> [128, nbtot*VW]; col D is the ones
        # column whose PV output is the softmax denominator
        vp_i = np.zeros((128, nbtot, VW), BF16)
        vp_i[:, :, 0:D] = vc_sel[:, :, i, :].transpose(1, 0, 2)
        vp_i[:, :, D] = BF16(1.0)
        m = {
            "hidT": hidT,
            "wq": _pack_w(Wq[:, i * G * D : (i + 1) * G * D]),
            "wkv": _pack_w(
                np.concatenate(
                    [Wk[:, i * D : (i + 1) * D], Wv[:, i * D : (i + 1) * D]], axis=1
                )
            ),
            "wo": np.ascontiguousarray(
                Wo[i * G * D : (i + 1) * G * D, :]
                .reshape(G, D, HID)
                .transpose(1, 0, 2)
                .reshape(128, G * HID)
            ).astype(BF16),
            "cssn": cssn,
            "kp": kp_i,
            "vp": vp_i.reshape(128, nbtot * VW),
            "maskd": maskd,
            "identd": np.eye(BS, dtype=np.float32),
        }
        if apply_qw:
            m["qw"] = np.ascontiguousarray(qnw.reshape(1, D))
        if apply_kw:
            m["kw"] = np.ascontiguousarray(knw.reshape(1, D))
        in_maps.append(m)

    key = (tuple(int(x) for x in seq_lens), apply_qw, apply_kw)
    if key not in _prog_cache:
        _prog_cache[key] = _build_program(seq_lens, apply_qw, apply_kw)
    nc = _prog_cache[key]
    return nc, in_maps


def kernel_with_stats(trace=False, **inputs):
    from concourse.bass_utils import run_bass_kernel_spmd

    nc, in_maps = _prepare(inputs)
    res = run_bass_kernel_spmd(
        nc, in_maps, core_ids=list(range(NCORES)), trace=trace
    )
    out = np.zeros((BS, HID), np.float32)
    for r in res.results:
        out += r["outp"].astype(np.float32)
    return out.reshape(1, BS, HID), res


def kernel(**inputs):
    out, _ = kernel_with_stats(trace=False, **inputs)
    return out


# revision 19
# speedup vs baseline: 1.1586x; 1.0182x over previous
"""Paged GQA decode attention (1 token/seq) on 8 trn2 NeuronCores.

Sharding: tensor-parallel over the 8 KV heads. Core i owns KV head i and
its G=4 query heads: Wq/Wk/Wv column-sharded, Wo row-sharded; each core
produces a partial [BS, HID] output and the host sums the 8 partials.

v6: all matmul operands bf16 (half the HBM traffic of f32); K packed on
host as [d, tok] and V as [tok, blk, d] so every partition reads one
contiguous run; K and V of each group merged into ONE ~4 MB DMA (fewer
stream boundaries keeps all 16 SDMA engines fed); groups are whole
sequences, first-fit-decreasing, many-small-seq bins streamed first and
a single big sequence last so the post-stream tail is short; quadruple
buffering; exp in 16-block sub-chunks so PV starts early; V-row patches
ride SWDGE (gpsimd) to stay off the main HWDGE ring; output written
bf16 (host accumulates in f32).

Per-core dataflow (one Bass program, SPMD over cores via per-core inputs):
  A) kv = hidT.T @ Wkv then q = hidT.T @ Wq (PE, chunked weight DMAs);
     RMSNorm + RoPE on [BS, D] tiles (ACT/DVE), attn scale folded into
     q's inverse-rms; k path first so the cache patches unblock early.
  B) per group: one [K_g | V_g] DMA; stale cache slot at pos=T-1 gets
     its KT column (DVE) / V row (SWDGE DMA) overwritten with this
     step's k/v; S^T[tok, 4*nbg] per block (lhsT=K block, rhs=qT_b);
     exp per 16-block sub-chunk; per-seq tail mask;
     per-seq O^T[4, 130] += matmul(lhsT=e_blk, rhs=V_blk) with the V
     ones-column giving the softmax denominator.
  C) partial = attn^T.T @ Wo_chunk (PE), 4 chunks of 1024 cols, bf16 out.
"""

import numpy as np
import ml_dtypes

BF16 = ml_dtypes.bfloat16

HID, H, HKV, D = 4096, 32, 8, 128
BS, BLOCKS_PER_SEQ, BLOCK_SIZE = 32, 32, 128
G = H // HKV
EPS = 1e-6
NCORES = 8
KTILES = HID // 128  # contraction tiles for the projections
VW = D + 2  # V row width: 128 values + ones col (denominator) + pad
KVW = BLOCK_SIZE + VW  # columns per block in the merged K|V group layout
GMAX = 64  # max cache blocks per streamed group
EXPB = 16  # blocks per exp sub-chunk (pipelines exp/PV under the S matmuls)
WCHUNK = 16  # k-tiles per weight DMA chunk (phase A starts on first chunk)
OCH = 1024  # output columns per phase-C chunk

_prog_cache = {}


def _plan(nb):
    """Pack whole sequences into groups of <= GMAX blocks.

    The largest sequence is held out as its own FINAL group (1 seq =>
    minimal tail work); the rest go first-fit-decreasing, and bins are
    ordered most-sequences-first so per-seq finalize work lands early.
    Returns (pack_order, groups); groups are (start_block, [seq ids])."""
    idx = sorted(range(len(nb)), key=lambda b: -nb[b])
    fin = idx[0]
    bins = []  # [total_blocks, [seq ids]]
    for b in idx[1:]:
        for bin_ in bins:
            if bin_[0] + nb[b] <= GMAX:
                bin_[0] += nb[b]
                bin_[1].append(b)
                break
        else:
            bins.append([nb[b], [b]])
    bins.sort(key=lambda x: -len(x[1]))
    bins.append([nb[fin], [fin]])
    groups, order, start = [], [], 0
    for tot, seqs in bins:
        groups.append((start, seqs))
        order.extend(seqs)
        start += tot
    return order, groups


def _build_program(seq_lens, apply_qw, apply_kw):
    import concourse.bass as bass
    import concourse.tile as tile
    from concourse import bacc, mybir

    f32 = mybir.dt.float32
    bf16 = mybir.dt.bfloat16
    AF = mybir.ActivationFunctionType

    nb = [(int(t) + BLOCK_SIZE - 1) // BLOCK_SIZE for t in seq_lens]
    nbtot = sum(nb)
    _, groups = _plan(nb)

    nc = bacc.Bacc("TRN2", target_bir_lowering=False)
    hidT = nc.dram_tensor("hidT", [128, KTILES * BS], bf16, kind="ExternalInput")
    wq = nc.dram_tensor("wq", [128, KTILES * G * D], bf16, kind="ExternalInput")
    wkv = nc.dram_tensor("wkv", [128, KTILES * 2 * D], bf16, kind="ExternalInput")
    wo = nc.dram_tensor("wo", [128, G * HID], bf16, kind="ExternalInput")
    cssn = nc.dram_tensor("cssn", [BS, 2 * D], f32, kind="ExternalInput")
    kvp = nc.dram_tensor("kvp", [128, nbtot * KVW], bf16, kind="ExternalInput")
    maskd = nc.dram_tensor("maskd", [128, 128], f32, kind="ExternalInput")
    identd = nc.dram_tensor("identd", [BS, BS], f32, kind="ExternalInput")
    if apply_qw:
        qw = nc.dram_tensor("qw", [1, D], f32, kind="ExternalInput")
    if apply_kw:
        kw = nc.dram_tensor("kw", [1, D], f32, kind="ExternalInput")
    outp = nc.dram_tensor("outp", [BS, HID], bf16, kind="ExternalOutput")

    with tile.TileContext(nc) as tc:
        with tc.tile_pool(name="sb", bufs=1) as sb, tc.tile_pool(
            name="smalls", bufs=4
        ) as smalls:
            # SP-ring DMAs first: the phase-A critical path; wkv/wq are
            # chunked so the projection matmuls start on the first chunk
            hid_sb = sb.tile([128, KTILES * BS], bf16, name="hid_sb")
            nc.sync.dma_start(out=hid_sb, in_=hidT[:, :])
            wkv_sb = sb.tile([128, KTILES * 2 * D], bf16, name="wkv_sb")
            for c0 in range(0, KTILES, WCHUNK):
                nc.sync.dma_start(
                    out=wkv_sb[:, c0 * 2 * D : (c0 + WCHUNK) * 2 * D],
                    in_=wkv[:, c0 * 2 * D : (c0 + WCHUNK) * 2 * D],
                )
            wq_sb = sb.tile([128, KTILES * G * D], bf16, name="wq_sb", tag="bigw")
            for c0 in range(0, KTILES, WCHUNK):
                nc.sync.dma_start(
                    out=wq_sb[:, c0 * G * D : (c0 + WCHUNK) * G * D],
                    in_=wq[:, c0 * G * D : (c0 + WCHUNK) * G * D],
                )
            # maskM[p, c] = 1.0 if p < c else 0.0 (tail masks for all Tmod)
            maskM = sb.tile([128, 128], f32, name="maskM")
            nc.sync.dma_start(out=maskM, in_=maskd[:, :])
            ident = sb.tile([BS, BS], f32, name="ident")
            nc.sync.dma_start(out=ident, in_=identd[:, :])
            # ACT-ring DMA: cos/sin (needed by norm_rope soon after wkv);
            # the streams stay on the SP ring — streaming DMAs on the
            # ACT ring serialize with exp (engine head-of-line blocking)
            cssn_sb = sb.tile([BS, 2 * D], f32, name="cssn_sb")
            nc.scalar.dma_start(out=cssn_sb, in_=cssn[:, :])
            cos_sb = cssn_sb[:, 0:D]
            sin_sb = cssn_sb[:, D : 2 * D]

            norm_w_sb = {}
            for flag, name, dram in (
                (apply_qw, "qw_sb", qw if apply_qw else None),
                (apply_kw, "kw_sb", kw if apply_kw else None),
            ):
                if flag:
                    t = sb.tile([BS, D], f32, name=name)
                    src = dram[:, :]
                    bcast = bass.AP(
                        tensor=src.tensor,
                        offset=src.offset,
                        ap=[[0, BS], list(src.ap[-1])],
                    )
                    nc.sync.dma_start(out=t, in_=bcast)
                    norm_w_sb[name] = t

            eps_q = sb.tile([BS, 1], f32, name="eps_q")
            nc.vector.memset(eps_q, float(D) * EPS)
            eps_k = sb.tile([BS, 1], f32, name="eps_k")
            nc.vector.memset(eps_k, EPS)

            qr_sb = sb.tile([BS, G * D], f32, name="qr_sb")
            kr_sb = sb.tile([BS, D], f32, name="kr_sb")
            vbf = sb.tile([BS, D], bf16, name="vbf")
            qT_sb = sb.tile([128, G * BS], bf16, name="qT_sb")
            kTbf = sb.tile([128, BS], bf16, name="kTbf")
            attn_T = sb.tile([128, G * BS], bf16, name="attn_T")

            with tc.tile_pool(name="psA", bufs=1, space="PSUM") as psA:
                q_ps = psA.tile([BS, G * D], f32, name="q_ps")
                kv_ps = psA.tile([BS, 2 * D], f32, name="kv_ps")
                last = KTILES - 1
                # all kv matmuls first: they only need wkv, so the k/v
                # path (and the cache patches) don't wait on the wq DMA
                for t in range(KTILES):
                    nc.tensor.matmul(
                        kv_ps,
                        hid_sb[:, t * BS : (t + 1) * BS],
                        wkv_sb[:, t * 2 * D : (t + 1) * 2 * D],
                        start=(t == 0), stop=(t == last),
                    )
                for t in range(KTILES):
                    nc.tensor.matmul(
                        q_ps,
                        hid_sb[:, t * BS : (t + 1) * BS],
                        wq_sb[:, t * G * D : (t + 1) * G * D],
                        start=(t == 0), stop=(t == last),
                    )
                k_ps = kv_ps[:, 0:D]
                v_ps = kv_ps[:, D : 2 * D]

                def norm_rope(src_ps, dst, head_cnt, is_q):
                    w_sb = norm_w_sb.get("qw_sb" if is_q else "kw_sb")
                    for h in range(head_cnt):
                        xin = src_ps[:, h * D : (h + 1) * D]
                        scratch = smalls.tile([BS, D], f32, name="scratch", tag="scr")
                        ssq = smalls.tile([BS, 1], f32, name="ssq", tag="ssq")
                        nc.scalar.activation(scratch, xin, AF.Square, accum_out=ssq)
                        s = smalls.tile([BS, 1], f32, name="s", tag="s")
                        if is_q:
                            # s = sqrt(sum(q^2) + D*eps): 1/s folds in the
                            # attention scale D**-0.5 on top of the rms norm
                            nc.scalar.activation(s, ssq, AF.Sqrt, bias=eps_q, scale=1.0)
                        else:
                            nc.scalar.activation(s, ssq, AF.Sqrt, bias=eps_k, scale=1.0 / D)
                        inv = smalls.tile([BS, 1], f32, name="inv", tag="inv")
                        nc.vector.reciprocal(inv, s)
                        xn = smalls.tile([BS, D], f32, name="xn", tag="xn")
                        nc.scalar.activation(xn, xin, AF.Copy, scale=inv)
                        if w_sb is not None:
                            nc.vector.tensor_mul(xn, xn, w_sb)
                        rot = smalls.tile([BS, D], f32, name="rot", tag="rot")
                        nc.scalar.mul(rot[:, 0 : D // 2], xn[:, D // 2 : D], -1.0)
                        nc.scalar.copy(rot[:, D // 2 : D], xn[:, 0 : D // 2])
                        t1 = smalls.tile([BS, D], f32, name="t1", tag="t1")
                        nc.vector.tensor_mul(t1, xn, cos_sb)
                        nc.vector.tensor_mul(rot, rot, sin_sb)
                        nc.vector.tensor_add(dst[:, h * D : (h + 1) * D], t1, rot)

                # k/v first: the per-seq cache patches gate on kTbf/vbf
                nc.vector.tensor_copy(vbf, v_ps)
                norm_rope(k_ps, kr_sb, 1, False)

                with tc.tile_pool(name="psT", bufs=2, space="PSUM") as psT:
                    tpk = psT.tile([128, BS], f32, name="tpk", tag="tp")
                    nc.tensor.transpose(tpk, kr_sb, ident)
                    nc.vector.tensor_copy(kTbf, tpk)
                    norm_rope(q_ps, qr_sb, G, True)
                    for h in range(G):
                        tp = psT.tile([128, BS], f32, name="tp", tag="tp")
                        nc.tensor.transpose(
                            tp, qr_sb[:, h * D : (h + 1) * D], ident
                        )
                        nc.vector.tensor_copy(qT_sb[:, h * BS : (h + 1) * BS], tp)

            # Wo reuses Wq's SBUF slot; DMA can start once phase A mms done.
            # Host pre-interleaves Wo as [chunk][head][OCH] so each phase-C
            # chunk is one contiguous slab.
            wo_sb = sb.tile([128, G * HID], bf16, name="wo_sb", tag="bigw")
            for c in range(HID // OCH):
                nc.sync.dma_start(
                    out=wo_sb[:, c * G * OCH : (c + 1) * G * OCH],
                    in_=wo[:, c * G * OCH : (c + 1) * G * OCH],
                )

            qT3 = qT_sb.rearrange("p (h c) -> p h c", c=BS)
            attn3 = attn_T.rearrange("p (h c) -> p h c", c=BS)

            with tc.tile_pool(name="psB", bufs=1, space="PSUM") as psB:
                for gi, (g0, seqs) in enumerate(groups):
                    nbg = sum(nb[b] for b in seqs)
                    kv = sb.tile(
                        [128, nbg * KVW], bf16, name=f"kv{gi}", tag="kvall",
                        bufs=4,
                    )
                    nc.sync.dma_start(
                        out=kv, in_=kvp[:, g0 * KVW : (g0 + nbg) * KVW]
                    )
                    kt = kv[:, 0 : nbg * BLOCK_SIZE]
                    vn3 = kv[:, nbg * BLOCK_SIZE : nbg * KVW].rearrange(
                        "p (n v) -> p n v", v=VW
                    )
                    # new token's k/v replace the stale cache slot
                    lb = 0
                    for b in seqs:
                        T = int(seq_lens[b])
                        r = (T - 1) % BLOCK_SIZE
                        le = lb + nb[b] - 1  # local index of seq's last block
                        nc.vector.tensor_copy(
                            kt[:, le * BLOCK_SIZE + r : le * BLOCK_SIZE + r + 1],
                            kTbf[:, b : b + 1],
                        )
                        nc.gpsimd.dma_start(
                            out=vn3[r : r + 1, le, 0:D], in_=vbf[b : b + 1, :]
                        )
                        lb += nb[b]

                    stp = psB.tile([128, 4 * nbg], f32, name=f"stp{gi}", tag="stp",
                                   bufs=2)
                    lb = 0
                    for b in seqs:
                        qTb = qT3[:, :, b]
                        for j in range(nb[b]):
                            jl = lb + j
                            nc.tensor.matmul(
                                stp[:, 4 * jl : 4 * jl + 4],
                                kt[:, jl * BLOCK_SIZE : (jl + 1) * BLOCK_SIZE],
                                qTb,
                                start=True,
                                stop=True,
                            )
                        lb += nb[b]
                    e = sb.tile([128, 4 * nbg], bf16, name=f"e{gi}", tag="e", bufs=2)
                    # exp in sub-chunks so PV starts before all S mms finish
                    for c0 in range(0, nbg, EXPB):
                        c1 = min(c0 + EXPB, nbg)
                        nc.scalar.activation(
                            e[:, 4 * c0 : 4 * c1], stp[:, 4 * c0 : 4 * c1], AF.Exp
                        )
                    lb = 0
                    for b in seqs:
                        tmod = int(seq_lens[b]) % BLOCK_SIZE
                        le = lb + nb[b] - 1
                        if tmod != 0:
                            nc.vector.tensor_scalar_mul(
                                e[:, 4 * le : 4 * le + 4],
                                e[:, 4 * le : 4 * le + 4],
                                maskM[:, tmod : tmod + 1],
                            )
                        lb += nb[b]

                    lb = 0
                    for b in seqs:
                        ot_ps = psB.tile([4, VW], f32, name=f"ot{b}", tag="ot", bufs=2)
                        for j in range(nb[b]):
                            jl = lb + j
                            nc.tensor.matmul(
                                ot_ps,
                                e[:, 4 * jl : 4 * jl + 4],
                                vn3[:, jl, :],
                                start=(j == 0),
                                stop=(j == nb[b] - 1),
                            )
                        lb += nb[b]
                        rec = smalls.tile([4, 1], f32, name=f"rec{b}", tag="rec")
                        nc.vector.reciprocal(rec, ot_ps[:, D : D + 1])
                        o_sb = smalls.tile([4, D], f32, name=f"o{b}", tag="o")
                        nc.vector.tensor_scalar_mul(o_sb, ot_ps[:, 0:D], rec)
                        tp2 = psB.tile([128, 4], f32, name=f"tp2_{b}", tag="tp2",
                                       bufs=2)
                        nc.tensor.transpose(tp2, o_sb, ident[:4, :4])
                        nc.vector.tensor_copy(attn3[:, :, b], tp2)

            with tc.tile_pool(name="psC", bufs=2, space="PSUM") as psC:
                for c in range(HID // OCH):
                    oc = psC.tile([BS, OCH], f32, name=f"oc{c}", tag="oc")
                    for h in range(G):
                        for u in range(OCH // 512):
                            nc.tensor.matmul(
                                oc[:, u * 512 : (u + 1) * 512],
                                attn_T[:, h * BS : (h + 1) * BS],
                                wo_sb[:, c * G * OCH + h * OCH + u * 512 :
                                      c * G * OCH + h * OCH + (u + 1) * 512],
                                start=(h == 0), stop=(h == G - 1),
                            )
                    ocs = sb.tile([BS, OCH], bf16, name=f"ocs{c}", tag="ocs", bufs=2)
                    nc.vector.tensor_copy(ocs, oc)
                    nc.sync.dma_start(out=outp[:, c * OCH : (c + 1) * OCH], in_=ocs)

    nc.compile()
    return nc


def _pack_w(w):
    # [4096, C] -> [128, KTILES*C]; sbuf[p, t*C + c] == w[t*128 + p, c]
    C = w.shape[1]
    return np.ascontiguousarray(
        w.reshape(KTILES, 128, C).transpose(1, 0, 2).reshape(128, KTILES * C)
    ).astype(BF16)


def _prepare(inputs):
    hs = np.ascontiguousarray(np.asarray(inputs["hidden_states"], np.float32)[0])
    Wq = np.asarray(inputs["Wq"], np.float32)
    Wk = np.asarray(inputs["Wk"], np.float32)
    Wv = np.asarray(inputs["Wv"], np.float32)
    Wo = np.asarray(inputs["Wo"], np.float32)
    cos_t = np.asarray(inputs["cos"], np.float32)[0]
    sin_t = np.asarray(inputs["sin"], np.float32)[0]
    qnw = np.asarray(inputs["q_norm_w"], np.float32)
    knw = np.asarray(inputs["k_norm_w"], np.float32)
    key_cache = np.asarray(inputs["key_cache"], np.float32)
    value_cache = np.asarray(inputs["value_cache"], np.float32)
    seq_lens = np.asarray(inputs["seq_lens_k"]).astype(np.int64)
    bt = np.asarray(inputs["block_table"]).astype(np.int64)

    apply_qw = not np.all(qnw == 1.0)
    apply_kw = not np.all(knw == 1.0)

    nb = [(int(t) + BLOCK_SIZE - 1) // BLOCK_SIZE for t in seq_lens]
    nbtot = sum(nb)
    order, groups = _plan(nb)
    blocks = np.concatenate([bt[b, : nb[b]] for b in order])

    hidT = np.ascontiguousarray(
        hs.T.reshape(KTILES, 128, BS).transpose(1, 0, 2).reshape(128, KTILES * BS)
    ).astype(BF16)

    # gather valid blocks once, cast to bf16 once: [nbtot, 128, HKV, D]
    kc_sel = key_cache[blocks].astype(BF16)
    vc_sel = value_cache[blocks].astype(BF16)

    cssn = np.ascontiguousarray(np.concatenate([cos_t, sin_t], axis=1))
    maskd = (np.arange(128)[:, None] < np.arange(128)[None, :]).astype(np.float32)
    gsizes = [sum(nb[b] for b in seqs) for _, seqs in groups]

    in_maps = []
    for i in range(NCORES):
        # merged [K_g | V_g] layout per group:
        #   K part [d, blk, tok]; V part [tok, blk, d+pad] with ones col D
        kT_all = np.ascontiguousarray(kc_sel[:, :, i, :].transpose(2, 0, 1))
        v_all = vc_sel[:, :, i, :].transpose(1, 0, 2)
        parts = []
        goff = 0
        for nbg in gsizes:
            kpart = kT_all[:, goff : goff + nbg, :].reshape(128, nbg * BLOCK_SIZE)
            vpart = np.zeros((128, nbg, VW), BF16)
            vpart[:, :, 0:D] = v_all[:, goff : goff + nbg, :]
            vpart[:, :, D] = BF16(1.0)
            parts.append(kpart)
            parts.append(vpart.reshape(128, nbg * VW))
            goff += nbg
        kvp_i = np.ascontiguousarray(np.concatenate(parts, axis=1))

        # Wo interleaved as [chunk][head][OCH] to match phase C's layout
        wo_i = (
            Wo[i * G * D : (i + 1) * G * D, :]
            .reshape(G, D, HID // OCH, OCH)
            .transpose(1, 2, 0, 3)
            .reshape(128, G * HID)
        )
        m = {
            "hidT": hidT,
            "wq": _pack_w(Wq[:, i * G * D : (i + 1) * G * D]),
            "wkv": _pack_w(
                np.concatenate(
                    [Wk[:, i * D : (i + 1) * D], Wv[:, i * D : (i + 1) * D]], axis=1
                )
            ),
            "wo": np.ascontiguousarray(wo_i).astype(BF16),
            "cssn": cssn,
            "kvp": kvp_i,
            "maskd": maskd,
            "identd": np.eye(BS, dtype=np.float32),
        }
        if apply_qw:
            m["qw"] = np.ascontiguousarray(qnw.reshape(1, D))
        if apply_kw:
            m["kw"] = np.ascontiguousarray(knw.reshape(1, D))
        in_maps.append(m)

    key = (tuple(int(x) for x in seq_lens), apply_qw, apply_kw)
    if key not in _prog_cache:
        _prog_cache[key] = _build_program(seq_lens, apply_qw, apply_kw)
    nc = _prog_cache[key]
    return nc, in_maps


def kernel_with_stats(trace=False, **inputs):
    from concourse.bass_utils import run_bass_kernel_spmd

    nc, in_maps = _prepare(inputs)
    res = run_bass_kernel_spmd(
        nc, in_maps, core_ids=list(range(NCORES)), trace=trace
    )
    out = np.zeros((BS, HID), np.float32)
    for r in res.results:
        out += r["outp"].astype(np.float32)
    return out.reshape(1, BS, HID), res


def kernel(**inputs):
    out, _ = kernel_with_stats(trace=False, **inputs)
    return out
